# revision 23
# baseline (speedup 1.0000x reference)
"""nn_AdditiveAttention Trainium2 kernel (8 NeuronCores, SPMD data-parallel).

reference:
    q_proj = Q @ Wq                       [B, Lq, d_ff]
    k_proj = K @ Wk                       [B, Lk, d_ff]
    energy[b,q,k] = v . tanh(q_proj[b,q] + k_proj[b,k])
    energy = where(mask==0, -1e30, energy)
    attn = softmax(energy, axis=-1); context = attn @ V
    returns (context, attn)

Strategy:
  - Shard over (batch, query-block): core c -> batch c//4, queries 64*(c%4)..+64.
  - Host compacts keys by mask (masked keys get exactly-zero attention in the
    reference, so they are dropped); pads key count to a multiple of 32 with
    crafted rows whose k_proj = -sign(v)*3e4, making pad energies ~ -sum|v|
    (negligible in softmax) and pad V rows zero (no context contribution).
    Host also pre-shuffles each shard into its exact on-chip layout (partition-
    contiguous bf16) so input DMA runs at full HBM bandwidth.
  - Device: bf16 projections on TensorE (d-chunk-outer so DMA overlaps);
    per-(query, f-chunk) fused tanh(k_projT + per-partition bias) on ScalarE
    (the throughput floor: 1 elem/lane/cycle); M=1 col-group matmuls against v
    reduce over f into PSUM rows {0,32,64,96}; DVE evicts; DMA shuffles rows
    dense; per-32-query-half softmax (Exp+accum rowsum; tanh-bounded energies
    need no max subtraction), PE transpose, attn @ V — overlapped with the
    other half's tanh work.
"""
import sys
import numpy as np

sys.path.insert(0, "/opt/trn_rl_repo")

B, LQ_FULL, LK, DM, DF = 2, 256, 1024, 1024, 512
LQ = 64          # queries per core
NCORES = 8
NEG = -1e30

TRACE = False
LAST_RESULTS = None
_CACHE = {}


def _nsplits(x):
    if x <= 512:
        return [(0, 0, x)]
    h = (x // 2 + 15) // 16 * 16
    return [(0, 0, h), (1, h, x - h)]


def _make_tile_context(nc):
    import concourse.tile as tile
    from concourse.tile_scheduler import N_PROCS
    from concourse.vector_clock import ScopedClock, VectorClock

    class TileContext1W(tile.TileContext):
        # walrus here rejects instructions with >1 sync wait; split the final
        # drain into one single-wait drain per outstanding proc.
        def _drain_and_barrier(self, tick_clock, wait_clock):
            from concourse.tile_scheduler import PROC_NAMES
            gc = tick_clock.global_clock
            for p in range(N_PROCS):
                if gc[p] > 0 and ("DMA" in PROC_NAMES[p]
                                  or "Collect" in PROC_NAMES[p]):
                    d = self.nc.sync.drain()
                    vc = VectorClock(
                        [gc[i] if i == p else 0 for i in range(N_PROCS)]
                    )
                    wait_clock.add_sem_waits(d.ins, ScopedClock({None: vc}))
            self.nc.all_engine_barrier()
            assert self.sems is not None
            popped = self.nc._tile_sem_poison_stack.pop()
            assert popped is self._sem_poison
            self.nc.clear_and_free_semaphores(
                list(self.sems.allocated().values())
            )

    return TileContext1W(nc)


def _audit_multiwait(nc):
    bad = []
    for f in nc.m.functions:
        for bb in f.blocks:
            for ins in bb.instructions:
                w = ins.sync_info.on_wait if ins.sync_info else None
                if w and len(w) > 1:
                    bad.append((bb.name, ins.name, type(ins).__name__, len(w)))
    return bad


def _split_multiwaits(nc):
    """walrus codegen allows at most one sync wait per instruction; hoist
    extras onto standalone same-engine event-semaphore instructions."""
    import concourse.mybir as mybir

    n_split = 0
    for f in nc.m.functions:
        for bb in f.blocks:
            new = []
            changed = False
            for ins in bb.instructions:
                si = ins.sync_info
                w = list(si.on_wait) if si and si.on_wait else []
                if len(w) > 1:
                    changed = True
                    for i, sw in enumerate(w[:-1]):
                        ev = mybir.InstEventSemaphore(
                            name=f"{ins.name}_hw{i}", ins=[], outs=[])
                        ev.engine = ins.engine
                        ev.sync_info = mybir.SyncInfo(on_wait=[sw], on_update=[])
                        new.append(ev)
                        n_split += 1
                    si.on_wait = [w[-1]]
                new.append(ins)
            if changed:
                bb.instructions = new
    return n_split


def _build(KC):
    import concourse.bass as bass
    import concourse.mybir as mybir
    from concourse.masks import make_identity

    f32 = mybir.dt.float32
    bf16 = mybir.dt.bfloat16
    AF = mybir.ActivationFunctionType
    nkb = (KC + 127) // 128
    KCM = nkb * 128
    NS = _nsplits(KC)

    nc = bass.Bass("TRN2", target_bir_lowering=False, num_devices=NCORES)
    # inputs arrive pre-shuffled to partition-contiguous on-chip layout, bf16
    qT_ext = nc.dram_tensor("qT", [128, 8, LQ], bf16, kind="ExternalInput")
    kT_ext = nc.dram_tensor("kT", [128, 8, KC], bf16, kind="ExternalInput")
    vc_ext = nc.dram_tensor("vc", [128, nkb, DM], bf16, kind="ExternalInput")
    wq_ext = nc.dram_tensor("wq", [128, 8, DF], bf16, kind="ExternalInput")
    wk_ext = nc.dram_tensor("wk", [128, 8, DF], bf16, kind="ExternalInput")
    vsb_ext = nc.dram_tensor("vsb", [128, 4], bf16, kind="ExternalInput")
    out_ctx = nc.dram_tensor("out_ctx", [LQ, DM], f32, kind="ExternalOutput")
    out_attn = nc.dram_tensor("out_attn", [LQ, KC], f32, kind="ExternalOutput")

    tc = _make_tile_context(nc)
    with tc:
        with tc.tile_pool(name="const", bufs=1) as const, \
             tc.tile_pool(name="thi_p", bufs=3) as thip, \
             tc.tile_pool(name="tho_p", bufs=2) as thop, \
             tc.tile_pool(name="scat", bufs=3) as scatp, \
             tc.tile_pool(name="ps", bufs=4, space="PSUM") as psp:

            def pstile(pp, ff, nm):
                # one shared slot shape: 2 PSUM banks
                return psp.tile([128, 1024], f32, tag="A", name=nm)[:pp, :ff]

            # ---- input DMAs: few instructions with long contiguous rows
            # (4-8KB descriptors) so the HWDGE rings never starve
            kT_bf = const.tile([128, 8, KC], bf16, name="kT_bf")
            wk_bf = const.tile([128, 8, DF], bf16, name="wk_bf")
            for h in (0, 1):
                hs = slice(4 * h, 4 * h + 4)
                nc.sync.dma_start(kT_bf[:, hs, :], kT_ext[:, hs, :])
                nc.sync.dma_start(wk_bf[:, hs, :], wk_ext[:, hs, :])
            qT_bf = const.tile([128, 8, LQ], bf16, name="qT_bf")
            nc.sync.dma_start(qT_bf[:], qT_ext[:])
            wq_bf = const.tile([128, 8, DF], bf16, name="wq_bf")
            nc.sync.dma_start(wq_bf[:], wq_ext[:])
            v_bf = const.tile([128, 4], bf16, name="v_bf")
            nc.sync.dma_start(v_bf[:], vsb_ext[:])

            # ---- kpT: d-chunk OUTER so each arriving kT half is consumed
            # immediately (4 concurrent psum accumulators)
            kps = [psp.tile([128, 1024], f32, tag="A", name=f"kps{c}")[
                :].rearrange("p (b n) -> p b n", b=2) for c in range(4)]
            for dc in range(8):
                for c in range(4):
                    fs = slice(c * 128, (c + 1) * 128)
                    for bank, off, sz in NS:
                        nc.tensor.matmul(kps[c][:, bank, 0:sz],
                                         wk_bf[:, dc, fs],
                                         kT_bf[:, dc, off:off + sz],
                                         start=(dc == 0), stop=(dc == 7))
            kpT = []
            for c in range(4):
                t = const.tile([128, KC], bf16, name=f"kpT{c}")
                for bank, off, sz in NS:
                    nc.vector.tensor_copy(t[:, off:off + sz],
                                          kps[c][:, bank, 0:sz])
                kpT.append(t)

            # ---- qpT: all 4 f-chunks into one psum tile [128, 4*64]
            qps = pstile(128, 4 * LQ, "qps")
            for c in range(4):
                fs = slice(c * 128, (c + 1) * 128)
                for dc in range(8):
                    nc.tensor.matmul(qps[:, c * LQ:(c + 1) * LQ],
                                     wq_bf[:, dc, fs], qT_bf[:, dc, :],
                                     start=(dc == 0), stop=(dc == 7))
            qpT = const.tile([128, 4, LQ], f32, name="qpT")
            nc.vector.tensor_copy(qpT[:], qps[:])

            ident = const.tile([64, 64], bf16, name="ident")
            make_identity(nc, ident[:])

            e_dense = const.tile([LQ, KC], f32, name="e_dense")
            p_bf = const.tile([LQ, KC], bf16, name="p_bf")
            rowsum = const.tile([LQ, 1], f32, name="rowsum")
            rinv = const.tile([LQ, 1], f32, name="rinv")
            attn_f = const.tile([LQ, KC], f32, name="attn_f")
            ctx_sb = const.tile([LQ, DM], f32, name="ctx_sb")
            vc_bf = const.tile([128, nkb, DM], bf16, name="vc_bf")
            pTs = [const.tile([128, nkb, 32], bf16, name=f"pT{h}")
                   for h in (0, 1)]

            def group(g):
                # 4 queries = 16 f-chunk units in ONE ACT instruction: DVE
                # does the bias-add at 4x, ScalarE amortizes its bubble over
                # FD = 16*KC. Energy matmuls use two even bank-aligned halves.
                pe = psp.tile([128, 1024], f32, tag="A", name="pe")[
                    :].rearrange("p (b n) -> p b n", b=2)
                thi = thip.tile([128, 16, KC], bf16, tag="thi", name="thi")
                for u in range(16):
                    a, c = u // 4, u % 4
                    qi = 4 * g + a
                    nc.vector.tensor_scalar_add(
                        thi[:, u, :], kpT[c][:], qpT[:, c, qi:qi + 1])
                tho = thop.tile([128, 16, KC], bf16, tag="tho", name="tho")
                nc.scalar.activation(tho[:], thi[:], AF.Tanh)
                for u in range(16):
                    a, c = u // 4, u % 4
                    for bank, off, sz in NS:
                        nc.tensor.matmul(
                            pe[32 * a:32 * a + 1, bank, 0:sz],
                            v_bf[:, c:c + 1], tho[:, u, off:off + sz],
                            start=(c == 0), stop=(c == 3),
                            tile_position=(0, 32 * a))
                sc = scatp.tile([128, KC], f32, tag="scat", name="sc")
                for bank, off, sz in NS:
                    nc.vector.tensor_copy(sc[:, off:off + sz],
                                          pe[:, bank, 0:sz])
                src = sc[:].rearrange("(a b) n -> a b n", b=32)[:, 0, :]
                nc.sync.dma_start(e_dense[4 * g:4 * g + 4, :], src)

            def tail_half(h, ctxps):
                # softmax over 32 query rows at partitions 32h..32h+32.
                # No max-subtraction needed: |energy| <= sum|v| ~ 20.
                rows = slice(32 * h, 32 * h + 32)
                nc.scalar.activation(p_bf[rows, :], e_dense[rows, :], AF.Exp,
                                     accum_out=rowsum[rows, 0:1])
                nc.vector.reciprocal(rinv[rows], rowsum[rows])
                nc.vector.tensor_scalar_mul(attn_f[rows, :], p_bf[rows, :],
                                            rinv[rows, 0:1])
                nc.sync.dma_start(out_attn[rows, :], attn_f[rows, :])
                pT = pTs[h]
                if KC < KCM:
                    nc.gpsimd.memset(pT[:], 0.0)
                idn = ident[rows, 32 * h:32 * h + 32]
                for kb in range(nkb):
                    w = min(128, KC - kb * 128)
                    tp = psp.tile([128, 32], bf16, tag="A", name="tp")
                    nc.tensor.transpose(
                        tp[0:w, :], p_bf[rows, kb * 128:kb * 128 + w], idn)
                    nc.vector.tensor_copy(pT[0:w, kb, :], tp[0:w, :])
                for kb in range(nkb):
                    for hh in (0, 1):
                        nc.tensor.matmul(ctxps[rows, hh * 512:(hh + 1) * 512],
                                         pT[:, kb, :],
                                         vc_bf[:, kb, hh * 512:(hh + 1) * 512],
                                         start=(kb == 0), stop=(kb == nkb - 1))
                nc.vector.tensor_scalar_mul(ctx_sb[rows, :], ctxps[rows, :],
                                            rinv[rows, 0:1])
                nc.sync.dma_start(out_ctx[rows, :], ctx_sb[rows, :])

            # delay vc descriptor generation until kpT (hence kT/wk DMA)
            # is done: tiny WAW dep on vc_bf via a copy sourced from kpT[3]
            nc.vector.tensor_copy(vc_bf[0:1, 0, 0:2], kpT[3][0:1, 0:2])
            for kb in range(nkb):
                nc.gpsimd.dma_start(vc_bf[:, kb, :], vc_ext[:, kb, :])
            for g in range(8):
                group(g)
            for g in range(8, 11):
                group(g)
            ctxps0 = pstile(64, 1024, "ctxps0")
            tail_half(0, ctxps0)
            for g in range(11, 16):
                group(g)
            ctxps1 = pstile(64, 1024, "ctxps1")
            tail_half(1, ctxps1)

    _split_multiwaits(nc)
    bad = _audit_multiwait(nc)
    assert not bad, f"multi-wait instructions remain: {bad[:5]}"
    return nc


def _shuffle(x, inner):
    """[N*128, inner] row-major -> [128, N, inner] partition-contiguous bf16."""
    import ml_dtypes
    n = x.shape[0] // 128
    return np.ascontiguousarray(
        x.reshape(n, 128, inner).transpose(1, 0, 2).astype(ml_dtypes.bfloat16))


def kernel(Q, K, V, mask, Wq, Wk, v):
    global LAST_RESULTS
    import ml_dtypes
    from concourse.bass_utils import run_bass_kernel_spmd

    Q = np.asarray(Q, np.float32)
    K = np.asarray(K, np.float32)
    V = np.asarray(V, np.float32)
    mask = np.asarray(mask)
    Wq = np.asarray(Wq, np.float32)
    Wk = np.asarray(Wk, np.float32)
    v = np.asarray(v, np.float32)

    keep = [np.flatnonzero(mask[b] != 0) for b in range(B)]
    counts = [len(k) for k in keep]

    # Degenerate all-masked batch: reference softmax of uniform -1e30 rows ->
    # uniform weights. Handle on host (cannot occur for the graded input).
    host_batches = [b for b in range(B) if counts[b] == 0]

    KC = max(32, ((max(counts) + 15) // 16) * 16)
    KC = min(KC, LK)
    nkb = (KC + 127) // 128
    KCM = nkb * 128

    # pad keys: k_proj row = -sign(v)*3e4 => tanh saturates to -sign(v)
    # => energy = -sum|v| (minimal possible), negligible after exp.
    t = -np.sign(v) * 3.0e4
    t[t == 0] = -3.0e4
    x_pad = Wk @ np.linalg.solve(Wk.T @ Wk, t)  # min-norm soln of Wk^T x = t

    wq_in = _shuffle(Wq, DF)
    wk_in = _shuffle(Wk, DF)
    vsb_in = np.ascontiguousarray(
        v.reshape(4, 128).T.astype(ml_dtypes.bfloat16))

    batch_data = {}
    for b in range(B):
        npad = KC - counts[b]
        Kc = np.concatenate(
            [K[b][keep[b]], np.tile(x_pad[None, :], (npad, 1))], axis=0)
        Vc = np.concatenate(
            [V[b][keep[b]], np.zeros((KCM - counts[b], DM), np.float32)], axis=0)
        batch_data[b] = (
            _shuffle(np.ascontiguousarray(Kc.T), KC),      # [128, 8, KC]
            _shuffle(Vc, DM),                              # [128, nkb, DM]
        )
    in_maps = []
    for core in range(NCORES):
        b, qb = core // 4, core % 4
        kT_in, vc_in = batch_data[b]
        qT_in = _shuffle(
            np.ascontiguousarray(Q[b, qb * LQ:(qb + 1) * LQ].T), LQ)
        in_maps.append({
            "qT": qT_in, "kT": kT_in, "vc": vc_in,
            "wq": wq_in, "wk": wk_in, "vsb": vsb_in,
        })

    if KC not in _CACHE:
        _CACHE[KC] = _build(KC)
    nc = _CACHE[KC]

    kwargs = {}
    if TRACE:
        kwargs = dict(trace=True, trace_cores=[0])
    res = run_bass_kernel_spmd(nc, in_maps, core_ids=list(range(NCORES)), **kwargs)
    LAST_RESULTS = res

    context = np.zeros((B, LQ_FULL, DM), np.float32)
    attn = np.zeros((B, LQ_FULL, LK), np.float32)
    for core in range(NCORES):
        b, qb = core // 4, core % 4
        qs = slice(qb * LQ, (qb + 1) * LQ)
        r = res.results[core]
        context[b, qs] = r["out_ctx"]
        attn[b, qs][:, keep[b]] = r["out_attn"][:, :counts[b]]

    for b in host_batches:
        attn[b] = 1.0 / LK
        context[b] = V[b].mean(axis=0, keepdims=True)

    return (context, attn)
